# revision 2
# baseline (speedup 1.0000x reference)
"""Blockwise butterfly rotation (nn_BlockwiseButterflyRotation) - TRN2 Bass kernel.

Full inputs: x (4, 4096, 4096) f32, angles (16, 8, 128) f32.
Math: x is split into 16 independent 256-wide blocks; each block's rows are
rotated by an 8-stage butterfly. The composed per-block rotation is a dense
256x256 matrix C_b, so out = x @ blockdiag(C). The kernel builds C on-device
from the angles and runs the bulk work as PE matmuls in bf16 (tolerance is
2e-2; bf16 path lands ~4e-3).

Sharding: data-parallel over rows - x.reshape(16384, 4096) split into 8
contiguous shards of 2048 rows; angle-derived tensors replicated.

Per-core dataflow, per 128-row tile:
  SWDGE DMA in [128, 4096] f32 -> bf16 cast during DMA
  -> 32x PE transpose (bf16, 1 cyc/col) of 128x128 chunks -> PSUM bf16
  -> PSUM->SBUF copy (DVE/ACT split, 2x mode for bf16)
  -> 32x PE matmul bf16 (FWL weight loads): out[128, 256] += xT_chunk^T @ C
  -> PSUM->SBUF copy f32 -> HWDGE DMA out

C build (once per pass, overlapped with the first row tiles): two-level
butterfly factorization C[16g+u, 16w+v] = LT_g[u,v] * HT_v[g,w]; LT (stages
0-3) and HT (stages 4-7) built by applying 16x16 butterflies to identity
patterns with free-dim-only pairing on the DVE (4 fused ops per stage);
cos/sin via ScalarE Sin (cos = sin(x + pi/2)); HT's u-replication via 16
bf16 selector matmuls on the PE; the combine multiplies bf16 operands and
writes CT bf16 directly. HSB is kept in (kc, w, v) free order so every
combine operand is contiguous. Constant 0/1 init patterns ship as two small
constant inputs (f32 + bf16) so no init chain sits on the critical path.
"""
import math
import os

import numpy as np

from concourse import bacc, mybir, tile
from concourse.bass_utils import run_bass_kernel_spmd

F32 = mybir.dt.float32
BF16 = mybir.dt.bfloat16

DIM = 4096
NB = 16
BLOCK = 256
HALF_PI = math.pi / 2.0

N_CORES = 8
R_TOTAL = 4 * 4096
R_CORE = R_TOTAL // N_CORES  # 2048

# f32 consts: halfpi | LSinit | HSBinit
_CF_PI = 0         # [128, 1] pi/2
_CF_LS = 1         # [128, 512] LS init: delta(v == p%16), free (b, kc, v)
_CF_HSB = 513      # [128, 512] HSB init: delta(w == 8kc + p//16), free (kc, w, v)
_CF_COLS = 1025
# bf16 consts: identity | W_all
_CB_ID = 0         # [128, 128] identity (PE transpose operand)
_CB_W = 128        # [128, 2048] W_all: free (b, mg, mu), delta(p == 16 mg + b)
_CB_COLS = 2176

LAST_RESULT = None
_NC_CACHE = {}


def _build_consts():
    p = np.arange(128)
    cf = np.zeros((128, _CF_COLS), dtype=np.float32)
    cf[:, _CF_PI] = HALF_PI
    ls = np.zeros((128, 16, 2, 16), np.float32)
    ls[p, :, :, p % 16] = 1.0
    cf[:, _CF_LS:_CF_LS + 512] = ls.reshape(128, 512)
    hsb = np.zeros((128, 2, 16, 16), np.float32)
    for kc in range(2):
        hsb[:, kc, :, :] = (np.arange(16)[None, :] == (8 * kc + p // 16)[:, None])[:, :, None]
    cf[:, _CF_HSB:_CF_HSB + 512] = hsb.reshape(128, 512)

    cb = np.zeros((128, _CB_COLS), dtype=np.float32)
    cb[:, _CB_ID:_CB_ID + 128] = np.eye(128, dtype=np.float32)
    w = np.zeros((128, 16, 8, 16), np.float32)
    for b in range(16):
        for mg in range(8):
            w[16 * mg + b, b, mg, :] = 1.0
    cb[:, _CB_W:_CB_W + 2048] = w.reshape(128, 2048)
    cb = cb.astype(mybir.dt.np(BF16))
    return cf, cb


_CF, _CB = _build_consts()


def gather_angles(angles: np.ndarray) -> np.ndarray:
    """angles [16, 8, 128] f32 -> ang [128, 1536] f32 (angL 4x256 | angH 4x128).

    Pure gather (indexing only, no arithmetic) into the per-partition
    coefficient layouts the kernel's butterfly-stage APs iterate.
    """
    angles = np.asarray(angles)
    assert angles.shape == (NB, 8, 128)
    p = np.arange(128)
    out = np.empty((128, 1536), dtype=np.float32)
    # L stages: col layout (b:16, kc:2, vg, t) with angle idx 8g + sig*vg + t,
    # g = 8kc + p//16, replicated over u = p%16.
    for s in range(4):
        sig = 1 << s
        col = np.empty((128, 16, 2, 8), dtype=np.float32)
        for kc in range(2):
            g = 8 * kc + p // 16
            for vt in range(8):
                col[:, :, kc, vt] = angles[:, s, 8 * g + vt].T
        out[:, 256 * s:256 * (s + 1)] = col.reshape(128, 256)
    # H stages: verbatim angles[b, sp+4, :] replicated over g0 = p//16.
    for sp in range(4):
        out[:, 1024 + 128 * sp:1024 + 128 * (sp + 1)] = angles[p % 16, sp + 4, :]
    return out


def _stage_pair_last(nc, pool, data, n1, n2, sig, cos_ap, sin_ap, gps=False):
    """One butterfly stage on `data` viewed as [p, n1, n2, vg, 2, sig];
    pairs along the (vg, 2, sig) axis group. cos/sin APs iterate
    [p, n1, n2, vg, sig]. 6 elementwise ops; half on GPSIMD if gps."""
    ng = 8 // sig
    e2 = nc.gpsimd if gps else nc.vector
    v = data.rearrange("p (n1 n2 vg h t) -> p n1 n2 vg h t",
                       n1=n1, n2=n2, vg=ng, h=2, t=sig)
    a = v[:, :, :, :, 0, :]
    b_ = v[:, :, :, :, 1, :]
    half = n1 * n2 * 8
    t1 = pool.tile([128, half], F32, name="bt_t1", tag="bt_t1")
    t2 = pool.tile([128, half], F32, name="bt_t2", tag="bt_t2")
    t3 = pool.tile([128, half], F32, name="bt_t3", tag="bt_t3")
    t4 = pool.tile([128, half], F32, name="bt_t4", tag="bt_t4")
    tv = lambda t: t[:].rearrange("p (n1 n2 vg t) -> p n1 n2 vg t",
                                  n1=n1, n2=n2, vg=ng, t=sig)
    nc.vector.tensor_mul(tv(t1), a, cos_ap)
    nc.vector.tensor_mul(tv(t2), b_, sin_ap)
    e2.tensor_mul(tv(t3), a, sin_ap)
    e2.tensor_mul(tv(t4), b_, cos_ap)
    nc.vector.tensor_sub(a, tv(t1), tv(t2))
    e2.tensor_add(b_, tv(t3), tv(t4))


def _stage_pair_mid(nc, pool, data, sig, cosT, sinT, gps=False):
    """Butterfly on data [128, 512] viewed (kc, wg, h, t, v), pairing along
    the middle (w) axis; coeff tiles [128, 256] are already h-expanded in
    (wg, h, t, v) order, broadcast over kc only. 4 fused ops; half on
    GPSIMD if gps."""
    ng = 8 // sig
    e2 = nc.gpsimd if gps else nc.vector
    dv = data.rearrange("p (kc wg h t v) -> p kc wg h t v",
                        kc=2, wg=ng, h=2, t=sig, v=16)
    cv = cosT[:].rearrange("p (wg h t v) -> p wg h t v",
                           wg=ng, h=2, t=sig, v=16) \
        .unsqueeze(1).to_broadcast((128, 2, ng, 2, sig, 16))
    sv = sinT[:].rearrange("p (wg h t v) -> p wg h t v",
                           wg=ng, h=2, t=sig, v=16) \
        .unsqueeze(1).to_broadcast((128, 2, ng, 2, sig, 16))
    ct = pool.tile([128, 512], F32, name="bt_ct", tag="bt_ct")
    st = pool.tile([128, 512], F32, name="bt_st", tag="bt_st")
    ctv = ct[:].rearrange("p (kc wg h t v) -> p kc wg h t v",
                          kc=2, wg=ng, h=2, t=sig, v=16)
    stv = st[:].rearrange("p (kc wg h t v) -> p kc wg h t v",
                          kc=2, wg=ng, h=2, t=sig, v=16)
    nc.vector.tensor_mul(ctv, dv, cv)
    e2.tensor_mul(stv, dv, sv)
    a = dv[:, :, :, 0, :, :]
    b_ = dv[:, :, :, 1, :, :]
    nc.vector.tensor_sub(a, ctv[:, :, :, 0, :, :], stv[:, :, :, 1, :, :])
    e2.tensor_add(b_, stv[:, :, :, 0, :, :], ctv[:, :, :, 1, :, :])


def build_nc(R: int, repeat: int | None = None, repeat_scope: str = "main",
             gps: bool = True):
    """repeat: if set, wrap the kernel body in an on-device For_i that re-runs
    it `repeat` times on the same data (identical output; used by the timing
    harness to resolve per-pass time above the dispatch noise floor). For
    repeat_scope="all" the loop body holds TWO passes with double-buffered CT
    (so pass k+1's C build never stalls on pass k's matmuls) - repeat must be
    even. gps: offload half the butterfly-stage elementwise ops to GPSIMD."""
    import contextlib
    assert R % 128 == 0
    RT = R // 128
    nc = bacc.Bacc("TRN2", target_bir_lowering=False, debug=False)

    X = nc.dram_tensor("x", [R, DIM], F32, kind="ExternalInput").ap()
    ANG = nc.dram_tensor("ang", [128, 1536], F32, kind="ExternalInput").ap()
    CF = nc.dram_tensor("cf", [128, _CF_COLS], F32, kind="ExternalInput").ap()
    CB = nc.dram_tensor("cb", [128, _CB_COLS], BF16, kind="ExternalInput").ap()
    OUT = nc.dram_tensor("out", [R, DIM], F32, kind="ExternalOutput").ap()

    with tile.TileContext(nc) as tc:
        with tc.tile_pool(name="const", bufs=1) as cpool, \
             tc.tile_pool(name="build", bufs=1) as bpool, \
             tc.tile_pool(name="xin", bufs=4) as xpool, \
             tc.tile_pool(name="xt", bufs=4) as xtpool, \
             tc.tile_pool(name="outp", bufs=3) as opool, \
             tc.tile_pool(name="psR", bufs=2, space="PSUM") as psR, \
             tc.tile_pool(name="psT", bufs=2, space="PSUM") as psT, \
             tc.tile_pool(name="psO", bufs=2, space="PSUM") as psO:
            cf = cpool.tile([128, _CF_COLS], F32)
            nc.sync.dma_start(out=cf[:], in_=CF)
            cb = cpool.tile([128, _CB_COLS], BF16)
            nc.sync.dma_start(out=cb[:], in_=CB)
            halfpi = cf[:, _CF_PI:_CF_PI + 1]
            identB = cb[:, _CB_ID:_CB_ID + 128]

            def emit_build(CT):
                # ---------------- C build ----------------
                angsb = bpool.tile([128, 1536], F32, name="angsb", tag="angsb")
                nc.sync.dma_start(out=angsb[:], in_=ANG)

                # LS [p=(g0,u), (b, kc, v)], HSB [p=(g0,b), (kc, w, v)]
                LS = bpool.tile([128, 512], F32, name="LS", tag="LS")
                nc.vector.tensor_copy(LS[:], cf[:, _CF_LS:_CF_LS + 512])
                HSB = bpool.tile([128, 512], F32, name="HSB", tag="HSB")
                nc.scalar.copy(HSB[:], cf[:, _CF_HSB:_CF_HSB + 512])

                # HSB stages 4-7 first (they gate the longest downstream
                # chain). ACT writes cos/sin h-expanded in (wg, h, t, v).
                for sp in range(4):
                    sigp = 1 << sp
                    ng = 8 // sigp
                    cosT = bpool.tile([128, 256], F32, name="cosH", tag="cosH")
                    sinT = bpool.tile([128, 256], F32, name="sinH", tag="sinH")
                    asl = angsb[:, 1024 + 128 * sp:1024 + 128 * (sp + 1)] \
                        .rearrange("p (wg t v) -> p wg t v", wg=ng, t=sigp, v=16) \
                        .unsqueeze(2).to_broadcast((128, ng, 2, sigp, 16))
                    co = cosT[:].rearrange("p (wg h t v) -> p wg h t v",
                                           wg=ng, h=2, t=sigp, v=16)
                    so = sinT[:].rearrange("p (wg h t v) -> p wg h t v",
                                           wg=ng, h=2, t=sigp, v=16)
                    nc.scalar.activation(co, asl, mybir.ActivationFunctionType.Sin,
                                         bias=halfpi, scale=1.0)
                    nc.scalar.activation(so, asl, mybir.ActivationFunctionType.Sin,
                                         bias=0.0, scale=1.0)
                    _stage_pair_mid(nc, bpool, HSB[:], sigp, cosT, sinT, gps=gps)

                # LS stages 0-3
                for s in range(4):
                    sig = 1 << s
                    ng = 8 // sig
                    cosT = bpool.tile([128, 256], F32, name="cosL", tag="cosL")
                    sinT = bpool.tile([128, 256], F32, name="sinL", tag="sinL")
                    asl = angsb[:, 256 * s:256 * (s + 1)]
                    nc.scalar.activation(cosT[:], asl, mybir.ActivationFunctionType.Sin,
                                         bias=halfpi, scale=1.0)
                    nc.scalar.activation(sinT[:], asl, mybir.ActivationFunctionType.Sin,
                                         bias=0.0, scale=1.0)
                    cv = cosT[:].rearrange("p (b kc vg t) -> p b kc vg t",
                                           b=16, kc=2, vg=ng, t=sig)
                    sv = sinT[:].rearrange("p (b kc vg t) -> p b kc vg t",
                                           b=16, kc=2, vg=ng, t=sig)
                    _stage_pair_last(nc, bpool, LS[:], 16, 2, sig, cv, sv, gps=gps)

                # bf16 cast of HSB (cheap PE matmul operand)
                HSBb = bpool.tile([128, 512], BF16, name="HSBb", tag="HSBb")
                nc.vector.tensor_copy(HSBb[:], HSB[:])

                # Per block b: replicate HSB group-row b to all u-lanes via
                # the PE (HS_b[16g0+u, (kc,w,v)] = HSB[16g0+b, (kc,w,v)]),
                # then combine straight out of PSUM:
                # CT[p, b, kc, w, v] = LS[p, b, kc, v] * HS_b[p, kc, w, v].
                for b in range(16):
                    Wb = cb[:, _CB_W + 128 * b:_CB_W + 128 * (b + 1)]
                    psr = psR.tile([128, 512], F32, name="psr", tag="psr")
                    nc.tensor.matmul(psr[:], Wb, HSBb[:], start=True, stop=True)
                    o = CT[:, 512 * b:512 * (b + 1)] \
                        .rearrange("p (kc w v) -> p kc w v", kc=2, w=16, v=16)
                    i0 = LS[:, 32 * b:32 * (b + 1)] \
                        .rearrange("p (kc v) -> p kc v", kc=2, v=16) \
                        .unsqueeze(2).to_broadcast((128, 2, 16, 16))
                    i1 = psr[:].rearrange("p (kc w v) -> p kc w v",
                                          kc=2, w=16, v=16)
                    nc.vector.tensor_mul(o, i0, i1)

            def emit_main(CT):
                # ---------------- main loop ----------------
                for r in range(RT):
                    xin = xpool.tile([128, DIM], BF16, name="xin", tag="xin")
                    nc.gpsimd.dma_start(out=xin[:], in_=X[r * 128:(r + 1) * 128, :])

                    xT = xtpool.tile([128, DIM], BF16, name="xT", tag="xT")
                    for j in range(4):
                        pst = psT.tile([128, 1024], BF16, name="pst", tag="pst")
                        for q in range(8):
                            i = 8 * j + q
                            nc.tensor.transpose(
                                pst[:, 128 * q:128 * (q + 1)],
                                xin[:, 128 * i:128 * (i + 1)], identB)
                        if j % 2 == 0:
                            nc.vector.tensor_copy(xT[:, 1024 * j:1024 * (j + 1)], pst[:])
                        else:
                            nc.scalar.copy(xT[:, 1024 * j:1024 * (j + 1)], pst[:])

                    outt = opool.tile([128, DIM], F32, name="outt", tag="outt")
                    for jo in range(4):
                        pso = psO.tile([128, 1024], F32, name="pso", tag="pso")
                        for q in range(4):
                            b = 4 * jo + q
                            for kc in range(2):
                                i = 2 * b + kc
                                nc.tensor.matmul(
                                    pso[:, 256 * q:256 * (q + 1)],
                                    xT[:, 128 * i:128 * (i + 1)],
                                    CT[:, 256 * i:256 * (i + 1)],
                                    start=(kc == 0), stop=(kc == 1))
                        if jo % 2 == 0:
                            nc.vector.tensor_copy(
                                outt[:, 1024 * jo:1024 * (jo + 1)], pso[:])
                        else:
                            nc.scalar.copy(
                                outt[:, 1024 * jo:1024 * (jo + 1)], pso[:])
                    nc.sync.dma_start(out=OUT[r * 128:(r + 1) * 128, :], in_=outt[:])

            CT_A = cpool.tile([128, 8192], BF16)  # C: [p=k%128, (b, kc, w, v)]
            if repeat and repeat_scope == "all":
                assert repeat % 2 == 0, "repeat_scope='all' needs even repeat"
                CT_B = cpool.tile([128, 8192], BF16)
                with tc.For_i(0, repeat // 2, 1):
                    emit_build(CT_A)
                    emit_main(CT_A)
                    emit_build(CT_B)
                    emit_main(CT_B)
            elif repeat and repeat_scope == "main":
                emit_build(CT_A)
                with tc.For_i(0, repeat, 1):
                    emit_main(CT_A)
            else:
                emit_build(CT_A)
                emit_main(CT_A)

    nc.compile()
    return nc


def _get_nc():
    if "nc" not in _NC_CACHE:
        _NC_CACHE["nc"] = build_nc(R_CORE)
    return _NC_CACHE["nc"]


def make_in_maps(x: np.ndarray, angles: np.ndarray):
    xf = np.ascontiguousarray(x.reshape(R_TOTAL, DIM), dtype=np.float32)
    ang = gather_angles(np.asarray(angles, dtype=np.float32))
    return [
        {"x": np.ascontiguousarray(xf[c * R_CORE:(c + 1) * R_CORE]),
         "ang": ang, "cf": _CF, "cb": _CB}
        for c in range(N_CORES)
    ]


def kernel(x: np.ndarray, angles: np.ndarray) -> np.ndarray:
    global LAST_RESULT
    x = np.asarray(x)
    orig_shape = x.shape
    in_maps = make_in_maps(x, angles)
    nc = _get_nc()
    trace = os.environ.get("BFK_TRACE", "") == "1"
    res = run_bass_kernel_spmd(nc, in_maps, list(range(N_CORES)), trace=trace)
    LAST_RESULT = res
    out = np.concatenate([res.results[c]["out"] for c in range(N_CORES)], axis=0)
    return out.reshape(orig_shape).astype(x.dtype, copy=False)


# revision 3
# speedup vs baseline: 9.6965x; 9.6965x over previous
"""Blockwise butterfly rotation (nn_BlockwiseButterflyRotation) - TRN2 Bass kernel.

Full inputs: x (4, 4096, 4096) f32, angles (16, 8, 128) f32.
Math: x is split into 16 independent 256-wide blocks; each block's rows are
rotated by an 8-stage butterfly. The composed per-block rotation is a dense
256x256 matrix C_b, so out = x @ blockdiag(C). The kernel builds C on-device
from the angles and runs the bulk work as PE matmuls in bf16 (tolerance is
2e-2; bf16 path lands ~4e-3).

Sharding: data-parallel over rows - x.reshape(16384, 4096) split into 8
contiguous shards of 2048 rows; angle-derived tensors replicated.

Per-core dataflow, per 128-row tile:
  SWDGE DMA in [128, 4096] f32 -> bf16 cast during DMA
  -> 32x PE transpose (bf16, 1 cyc/col) of 128x128 chunks -> PSUM bf16
  -> PSUM->SBUF copy (DVE/ACT split, 2x mode for bf16)
  -> 32x PE matmul bf16 (FWL weight loads): out[128, 256] += xT_chunk^T @ C
  -> PSUM->SBUF copy f32 -> HWDGE DMA out

C build (once per pass, overlapped with the first row tiles): two-level
butterfly factorization C[16g+u, 16w+v] = LT_g[u,v] * HT_v[g,w]; LT (stages
0-3) and HT (stages 4-7) built by applying 16x16 butterflies to identity
patterns with free-dim-only pairing, split across DVE and GPSIMD; cos/sin
via ScalarE Sin (cos = sin(x + pi/2)); HT's u-replication via 16 bf16
selector matmuls on the PE; the combine reads the replication result
straight out of PSUM and writes CT bf16. HSB is kept in (kc, w, v) free
order so every combine operand is contiguous. Constant 0/1 init patterns
ship as two small constant inputs (f32 + bf16). In the timing harness's
repeat loop, CT is double-buffered across unrolled pass pairs so pass k+1's
build never stalls on pass k's matmuls.
"""
import math
import os

import numpy as np

from concourse import bacc, mybir, tile
from concourse.bass_utils import run_bass_kernel_spmd

F32 = mybir.dt.float32
BF16 = mybir.dt.bfloat16

DIM = 4096
NB = 16
BLOCK = 256
HALF_PI = math.pi / 2.0

N_CORES = 8
R_TOTAL = 4 * 4096
R_CORE = R_TOTAL // N_CORES  # 2048

# f32 consts: halfpi | LSinit | HSBinit
_CF_PI = 0         # [128, 1] pi/2
_CF_LS = 1         # [128, 512] LS init: delta(v == p%16), free (b, kc, v)
_CF_HSB = 513      # [128, 512] HSB init: delta(w == 8kc + p//16), free (kc, w, v)
_CF_COLS = 1025
# bf16 consts: identity | W_all
_CB_ID = 0         # [128, 128] identity (PE transpose operand)
_CB_W = 128        # [128, 2048] W_all: free (b, mg, mu), delta(p == 16 mg + b)
_CB_COLS = 2176

LAST_RESULT = None
_NC_CACHE = {}


def _build_consts():
    p = np.arange(128)
    cf = np.zeros((128, _CF_COLS), dtype=np.float32)
    cf[:, _CF_PI] = HALF_PI
    ls = np.zeros((128, 16, 2, 16), np.float32)
    ls[p, :, :, p % 16] = 1.0
    cf[:, _CF_LS:_CF_LS + 512] = ls.reshape(128, 512)
    hsb = np.zeros((128, 2, 16, 16), np.float32)
    for kc in range(2):
        hsb[:, kc, :, :] = (np.arange(16)[None, :] == (8 * kc + p // 16)[:, None])[:, :, None]
    cf[:, _CF_HSB:_CF_HSB + 512] = hsb.reshape(128, 512)

    cb = np.zeros((128, _CB_COLS), dtype=np.float32)
    cb[:, _CB_ID:_CB_ID + 128] = np.eye(128, dtype=np.float32)
    w = np.zeros((128, 16, 8, 16), np.float32)
    for b in range(16):
        for mg in range(8):
            w[16 * mg + b, b, mg, :] = 1.0
    cb[:, _CB_W:_CB_W + 2048] = w.reshape(128, 2048)
    cb = cb.astype(mybir.dt.np(BF16))
    return cf, cb


_CF, _CB = _build_consts()


def gather_angles(angles: np.ndarray) -> np.ndarray:
    """angles [16, 8, 128] f32 -> ang [128, 1536] f32 (angL 4x256 | angH 4x128).

    Pure gather (indexing only, no arithmetic) into the per-partition
    coefficient layouts the kernel's butterfly-stage APs iterate.
    """
    angles = np.asarray(angles)
    assert angles.shape == (NB, 8, 128)
    p = np.arange(128)
    out = np.empty((128, 1536), dtype=np.float32)
    # L stages: col layout (b:16, kc:2, vg, t) with angle idx 8g + sig*vg + t,
    # g = 8kc + p//16, replicated over u = p%16.
    for s in range(4):
        sig = 1 << s
        col = np.empty((128, 16, 2, 8), dtype=np.float32)
        for kc in range(2):
            g = 8 * kc + p // 16
            for vt in range(8):
                col[:, :, kc, vt] = angles[:, s, 8 * g + vt].T
        out[:, 256 * s:256 * (s + 1)] = col.reshape(128, 256)
    # H stages: verbatim angles[b, sp+4, :] replicated over g0 = p//16.
    for sp in range(4):
        out[:, 1024 + 128 * sp:1024 + 128 * (sp + 1)] = angles[p % 16, sp + 4, :]
    return out


def _stage_pair_last(nc, pool, data, n1, n2, sig, cos_ap, sin_ap, gps=False):
    """One butterfly stage on `data` viewed as [p, n1, n2, vg, 2, sig];
    pairs along the (vg, 2, sig) axis group. cos/sin APs iterate
    [p, n1, n2, vg, sig]. 6 elementwise ops; half on GPSIMD if gps."""
    ng = 8 // sig
    e2 = nc.gpsimd if gps else nc.vector
    v = data.rearrange("p (n1 n2 vg h t) -> p n1 n2 vg h t",
                       n1=n1, n2=n2, vg=ng, h=2, t=sig)
    a = v[:, :, :, :, 0, :]
    b_ = v[:, :, :, :, 1, :]
    half = n1 * n2 * 8
    t1 = pool.tile([128, half], F32, name="bt_t1", tag="bt_t1")
    t2 = pool.tile([128, half], F32, name="bt_t2", tag="bt_t2")
    t3 = pool.tile([128, half], F32, name="bt_t3", tag="bt_t3")
    t4 = pool.tile([128, half], F32, name="bt_t4", tag="bt_t4")
    tv = lambda t: t[:].rearrange("p (n1 n2 vg t) -> p n1 n2 vg t",
                                  n1=n1, n2=n2, vg=ng, t=sig)
    nc.vector.tensor_mul(tv(t1), a, cos_ap)
    nc.vector.tensor_mul(tv(t2), b_, sin_ap)
    e2.tensor_mul(tv(t3), a, sin_ap)
    e2.tensor_mul(tv(t4), b_, cos_ap)
    nc.vector.tensor_sub(a, tv(t1), tv(t2))
    e2.tensor_add(b_, tv(t3), tv(t4))


def _stage_pair_mid(nc, pool, data, sig, cosT, sinT, gps=False):
    """Butterfly on data [128, 512] viewed (kc, wg, h, t, v), pairing along
    the middle (w) axis; coeff tiles [128, 256] are already h-expanded in
    (wg, h, t, v) order, broadcast over kc only. 4 fused ops; half on
    GPSIMD if gps."""
    ng = 8 // sig
    e2 = nc.gpsimd if gps else nc.vector
    dv = data.rearrange("p (kc wg h t v) -> p kc wg h t v",
                        kc=2, wg=ng, h=2, t=sig, v=16)
    cv = cosT[:].rearrange("p (wg h t v) -> p wg h t v",
                           wg=ng, h=2, t=sig, v=16) \
        .unsqueeze(1).to_broadcast((128, 2, ng, 2, sig, 16))
    sv = sinT[:].rearrange("p (wg h t v) -> p wg h t v",
                           wg=ng, h=2, t=sig, v=16) \
        .unsqueeze(1).to_broadcast((128, 2, ng, 2, sig, 16))
    ct = pool.tile([128, 512], F32, name="bt_ct", tag="bt_ct")
    st = pool.tile([128, 512], F32, name="bt_st", tag="bt_st")
    ctv = ct[:].rearrange("p (kc wg h t v) -> p kc wg h t v",
                          kc=2, wg=ng, h=2, t=sig, v=16)
    stv = st[:].rearrange("p (kc wg h t v) -> p kc wg h t v",
                          kc=2, wg=ng, h=2, t=sig, v=16)
    nc.vector.tensor_mul(ctv, dv, cv)
    e2.tensor_mul(stv, dv, sv)
    a = dv[:, :, :, 0, :, :]
    b_ = dv[:, :, :, 1, :, :]
    nc.vector.tensor_sub(a, ctv[:, :, :, 0, :, :], stv[:, :, :, 1, :, :])
    e2.tensor_add(b_, stv[:, :, :, 0, :, :], ctv[:, :, :, 1, :, :])


def build_nc(R: int, repeat: int | None = None, repeat_scope: str = "main",
             gps: bool = True):
    """repeat: if set, wrap the kernel body in an on-device For_i that re-runs
    it `repeat` times on the same data (identical output; used by the timing
    harness to resolve per-pass time above the dispatch noise floor). For
    repeat_scope="all" the loop body holds TWO passes with double-buffered CT
    (so pass k+1's C build never stalls on pass k's matmuls) - repeat must be
    even. gps: offload half the butterfly-stage elementwise ops to GPSIMD."""
    import contextlib
    assert R % 128 == 0
    RT = R // 128
    nc = bacc.Bacc("TRN2", target_bir_lowering=False, debug=False)

    X = nc.dram_tensor("x", [R, DIM], F32, kind="ExternalInput").ap()
    ANG = nc.dram_tensor("ang", [128, 1536], F32, kind="ExternalInput").ap()
    CF = nc.dram_tensor("cf", [128, _CF_COLS], F32, kind="ExternalInput").ap()
    CB = nc.dram_tensor("cb", [128, _CB_COLS], BF16, kind="ExternalInput").ap()
    OUT = nc.dram_tensor("out", [R, DIM], F32, kind="ExternalOutput").ap()

    with tile.TileContext(nc) as tc:
        with tc.tile_pool(name="const", bufs=1) as cpool, \
             tc.tile_pool(name="build", bufs=1) as bpool, \
             tc.tile_pool(name="xin", bufs=4) as xpool, \
             tc.tile_pool(name="xt", bufs=4) as xtpool, \
             tc.tile_pool(name="outp", bufs=3) as opool, \
             tc.tile_pool(name="psR", bufs=2, space="PSUM") as psR, \
             tc.tile_pool(name="psT", bufs=2, space="PSUM") as psT, \
             tc.tile_pool(name="psO", bufs=2, space="PSUM") as psO:
            cf = cpool.tile([128, _CF_COLS], F32)
            nc.sync.dma_start(out=cf[:], in_=CF)
            cb = cpool.tile([128, _CB_COLS], BF16)
            nc.sync.dma_start(out=cb[:], in_=CB)
            halfpi = cf[:, _CF_PI:_CF_PI + 1]
            identB = cb[:, _CB_ID:_CB_ID + 128]

            def emit_build(CT):
                # ---------------- C build ----------------
                angsb = bpool.tile([128, 1536], F32, name="angsb", tag="angsb")
                nc.sync.dma_start(out=angsb[:], in_=ANG)

                # LS [p=(g0,u), (b, kc, v)], HSB [p=(g0,b), (kc, w, v)]
                LS = bpool.tile([128, 512], F32, name="LS", tag="LS")
                nc.vector.tensor_copy(LS[:], cf[:, _CF_LS:_CF_LS + 512])
                HSB = bpool.tile([128, 512], F32, name="HSB", tag="HSB")
                nc.scalar.copy(HSB[:], cf[:, _CF_HSB:_CF_HSB + 512])

                # HSB stages 4-7 first (they gate the longest downstream
                # chain). ACT writes cos/sin h-expanded in (wg, h, t, v).
                for sp in range(4):
                    sigp = 1 << sp
                    ng = 8 // sigp
                    cosT = bpool.tile([128, 256], F32, name="cosH", tag="cosH")
                    sinT = bpool.tile([128, 256], F32, name="sinH", tag="sinH")
                    asl = angsb[:, 1024 + 128 * sp:1024 + 128 * (sp + 1)] \
                        .rearrange("p (wg t v) -> p wg t v", wg=ng, t=sigp, v=16) \
                        .unsqueeze(2).to_broadcast((128, ng, 2, sigp, 16))
                    co = cosT[:].rearrange("p (wg h t v) -> p wg h t v",
                                           wg=ng, h=2, t=sigp, v=16)
                    so = sinT[:].rearrange("p (wg h t v) -> p wg h t v",
                                           wg=ng, h=2, t=sigp, v=16)
                    nc.scalar.activation(co, asl, mybir.ActivationFunctionType.Sin,
                                         bias=halfpi, scale=1.0)
                    nc.scalar.activation(so, asl, mybir.ActivationFunctionType.Sin,
                                         bias=0.0, scale=1.0)
                    _stage_pair_mid(nc, bpool, HSB[:], sigp, cosT, sinT, gps=gps)

                # LS stages 0-3
                for s in range(4):
                    sig = 1 << s
                    ng = 8 // sig
                    cosT = bpool.tile([128, 256], F32, name="cosL", tag="cosL")
                    sinT = bpool.tile([128, 256], F32, name="sinL", tag="sinL")
                    asl = angsb[:, 256 * s:256 * (s + 1)]
                    nc.scalar.activation(cosT[:], asl, mybir.ActivationFunctionType.Sin,
                                         bias=halfpi, scale=1.0)
                    nc.scalar.activation(sinT[:], asl, mybir.ActivationFunctionType.Sin,
                                         bias=0.0, scale=1.0)
                    cv = cosT[:].rearrange("p (b kc vg t) -> p b kc vg t",
                                           b=16, kc=2, vg=ng, t=sig)
                    sv = sinT[:].rearrange("p (b kc vg t) -> p b kc vg t",
                                           b=16, kc=2, vg=ng, t=sig)
                    _stage_pair_last(nc, bpool, LS[:], 16, 2, sig, cv, sv, gps=gps)

                # bf16 cast of HSB (cheap PE matmul operand)
                HSBb = bpool.tile([128, 512], BF16, name="HSBb", tag="HSBb")
                nc.vector.tensor_copy(HSBb[:], HSB[:])

                # Per block b: replicate HSB group-row b to all u-lanes via
                # the PE (HS_b[16g0+u, (kc,w,v)] = HSB[16g0+b, (kc,w,v)]),
                # then combine straight out of PSUM:
                # CT[p, b, kc, w, v] = LS[p, b, kc, v] * HS_b[p, kc, w, v].
                for b in range(16):
                    Wb = cb[:, _CB_W + 128 * b:_CB_W + 128 * (b + 1)]
                    psr = psR.tile([128, 512], F32, name="psr", tag="psr")
                    nc.tensor.matmul(psr[:], Wb, HSBb[:], start=True, stop=True)
                    o = CT[:, 512 * b:512 * (b + 1)] \
                        .rearrange("p (kc w v) -> p kc w v", kc=2, w=16, v=16)
                    i0 = LS[:, 32 * b:32 * (b + 1)] \
                        .rearrange("p (kc v) -> p kc v", kc=2, v=16) \
                        .unsqueeze(2).to_broadcast((128, 2, 16, 16))
                    i1 = psr[:].rearrange("p (kc w v) -> p kc w v",
                                          kc=2, w=16, v=16)
                    nc.vector.tensor_mul(o, i0, i1)

            def emit_main(CT):
                # ---------------- main loop ----------------
                for r in range(RT):
                    xin = xpool.tile([128, DIM], BF16, name="xin", tag="xin")
                    nc.gpsimd.dma_start(out=xin[:], in_=X[r * 128:(r + 1) * 128, :])

                    xT = xtpool.tile([128, DIM], BF16, name="xT", tag="xT")
                    for j in range(4):
                        pst = psT.tile([128, 1024], BF16, name="pst", tag="pst")
                        for q in range(8):
                            i = 8 * j + q
                            nc.tensor.transpose(
                                pst[:, 128 * q:128 * (q + 1)],
                                xin[:, 128 * i:128 * (i + 1)], identB)
                        if j % 2 == 0:
                            nc.vector.tensor_copy(xT[:, 1024 * j:1024 * (j + 1)], pst[:])
                        else:
                            nc.scalar.copy(xT[:, 1024 * j:1024 * (j + 1)], pst[:])

                    outt = opool.tile([128, DIM], F32, name="outt", tag="outt")
                    for jo in range(4):
                        pso = psO.tile([128, 1024], F32, name="pso", tag="pso")
                        for q in range(4):
                            b = 4 * jo + q
                            for kc in range(2):
                                i = 2 * b + kc
                                nc.tensor.matmul(
                                    pso[:, 256 * q:256 * (q + 1)],
                                    xT[:, 128 * i:128 * (i + 1)],
                                    CT[:, 256 * i:256 * (i + 1)],
                                    start=(kc == 0), stop=(kc == 1))
                        if jo % 2 == 0:
                            nc.vector.tensor_copy(
                                outt[:, 1024 * jo:1024 * (jo + 1)], pso[:])
                        else:
                            nc.scalar.copy(
                                outt[:, 1024 * jo:1024 * (jo + 1)], pso[:])
                    nc.sync.dma_start(out=OUT[r * 128:(r + 1) * 128, :], in_=outt[:])

            CT_A = cpool.tile([128, 8192], BF16)  # C: [p=k%128, (b, kc, w, v)]
            if repeat and repeat_scope == "all":
                assert repeat % 2 == 0, "repeat_scope='all' needs even repeat"
                CT_B = cpool.tile([128, 8192], BF16)
                with tc.For_i(0, repeat // 2, 1):
                    emit_build(CT_A)
                    emit_main(CT_A)
                    emit_build(CT_B)
                    emit_main(CT_B)
            elif repeat and repeat_scope == "main":
                emit_build(CT_A)
                with tc.For_i(0, repeat, 1):
                    emit_main(CT_A)
            else:
                emit_build(CT_A)
                emit_main(CT_A)

    nc.compile()
    return nc


def _get_nc():
    if "nc" not in _NC_CACHE:
        _NC_CACHE["nc"] = build_nc(R_CORE)
    return _NC_CACHE["nc"]


def make_in_maps(x: np.ndarray, angles: np.ndarray):
    xf = np.ascontiguousarray(x.reshape(R_TOTAL, DIM), dtype=np.float32)
    ang = gather_angles(np.asarray(angles, dtype=np.float32))
    return [
        {"x": np.ascontiguousarray(xf[c * R_CORE:(c + 1) * R_CORE]),
         "ang": ang, "cf": _CF, "cb": _CB}
        for c in range(N_CORES)
    ]


def kernel(x: np.ndarray, angles: np.ndarray) -> np.ndarray:
    global LAST_RESULT
    x = np.asarray(x)
    orig_shape = x.shape
    in_maps = make_in_maps(x, angles)
    nc = _get_nc()
    trace = os.environ.get("BFK_TRACE", "") == "1"
    res = run_bass_kernel_spmd(nc, in_maps, list(range(N_CORES)), trace=trace)
    LAST_RESULT = res
    out = np.concatenate([res.results[c]["out"] for c in range(N_CORES)], axis=0)
    return out.reshape(orig_shape).astype(x.dtype, copy=False)
